# revision 26
# baseline (speedup 1.0000x reference)
"""Trainium2 Bass kernel for nn_AlchemicalModel (gnn_message_passing).

Strategy (v2):
  - Host (numpy): edge-basis features, per-atom spherical expansion via
    sorted segment-sum, power spectrum, layer norm, species-sorted atom
    sharding, readout.
  - Device (8 NeuronCores, SPMD): the dominant species-indexed 3-layer
    MLP. One species per core (2 cores per species) so weights are
    DMA'd once per core. Layers 1+2 run as fp8-e4m3 DoubleRow matmuls
    (two 128-K-tiles per instruction, 2x tensor throughput, half DMA),
    with power-of-2 weight scaling folded into the activation scale.
    Layer 3 (H2 -> 1) runs on-device so the output DMA is tiny.
    L2/L3 of supertile st-1 are issued after L1 of st so ScalarE silu
    latency never stalls the PE.
Self-contained: hardcodes all shapes; no sibling imports.
"""

import os
import numpy as np
import ml_dtypes

N_ATOMS = 16384
N_EDGES = 524288
N_MOL = 256
A = 4
S = 4
NMAX = 10
NSPH = 9
Q = A * NMAX
F = Q * Q * 3          # 4800
FU = 2460              # unique (l, q<=p) power-spectrum columns
FP = 2560              # FU padded to 20*128
KT1 = 20               # k-tiles for layer 1
CUTOFF = 5.0
AVG_ATOMS = 64.0
SCALE = 1.0
H1, H2 = 512, 512
NCORES = 8
FP8MAX = 240.0         # TRN fp8_e4m3 saturation

LAST_EXEC_NS = None

_COMPILED = {}


def _sph_l012(u):
    x, y, z = u[:, 0], u[:, 1], u[:, 2]
    c1 = 0.4886025119029199
    c2 = 1.0925484305920792
    return np.stack([
        np.full_like(x, 0.28209479177387814),
        c1 * y, c1 * z, c1 * x,
        c2 * x * y, c2 * y * z,
        0.31539156525252005 * (3.0 * z * z - 1.0),
        c2 * x * z,
        0.5462742152960396 * (x * x - y * y),
    ], axis=-1).astype(np.float32)


def _host_features(positions, numbers, edge_indices, U, gamma, beta):
    """Edge basis -> spherical expansion -> power spectrum -> layernorm.

    Returns x [N, F] float32 (normalized, WITHOUT gamma/beta applied).
    """
    pos = np.asarray(positions, np.float32)
    send = np.asarray(edge_indices[0], np.int64)
    recv = np.asarray(edge_indices[1], np.int64)
    rvec = pos[recv] - pos[send]                       # cells/offsets are zeros
    r = np.sqrt((rvec * rvec).sum(-1) + 1e-12).astype(np.float32)
    rhat = (rvec / r[:, None]).astype(np.float32)
    fc = (0.5 * (np.cos(np.pi * r / CUTOFF) + 1.0) * (r < CUTOFF)).astype(np.float32)
    mu = np.linspace(0.0, CUTOFF, NMAX, dtype=np.float32)
    sig = CUTOFF / NMAX
    R = np.exp(-((r[:, None] - mu) ** 2) / (2.0 * sig * sig)).astype(np.float32) * fc[:, None]
    Y = _sph_l012(rhat)                                # [E,9]
    RYf = (R[:, :, None] * Y[:, None, :]).reshape(N_EDGES, NMAX * NSPH)
    w = np.asarray(U, np.float32)[:, np.asarray(numbers, np.int64)[send]]  # [A,E]

    order = np.argsort(recv, kind="stable")
    recv_s = recv[order]
    starts = np.searchsorted(recv_s, np.arange(N_ATOMS))
    counts = np.bincount(recv, minlength=N_ATOMS)
    starts_c = np.minimum(starts, N_EDGES - 1)
    RYs = RYf[order]
    c = np.empty((N_ATOMS, A, NMAX * NSPH), np.float32)
    for a in range(A):
        z = w[a][order, None] * RYs
        ca = np.add.reduceat(z, starts_c, axis=0)
        ca[counts == 0] = 0.0
        c[:, a] = ca
    c = c.reshape(N_ATOMS, Q, NSPH)

    lblocks = [(0, 1, 1.0), (1, 4, 3.0), (4, 9, 5.0)]
    ps = np.empty((N_ATOMS, Q, Q, 3), np.float32)
    for li, (a0, b0, nl) in enumerate(lblocks):
        cb = c[:, :, a0:b0]
        ps[:, :, :, li] = np.matmul(cb, cb.transpose(0, 2, 1)) / np.sqrt(nl)
    ps = ps.reshape(N_ATOMS, F)

    mean = ps.mean(axis=-1, keepdims=True)
    var = ps.var(axis=-1, keepdims=True)
    psn = (ps - mean) / np.sqrt(var + 1e-5)
    return psn.astype(np.float32)


def _plan_shards(numbers):
    """One species per core (2 cores per species), species-sorted atoms.

    Returns (core_species[8], core_idx[8] each [sum(widths)] with -1 pad,
    widths). Widths are per-supertile atom counts: a narrow first tile so
    the pipeline fills with little DMA, 512-wide middles, and a narrow
    last tile to shorten the serial drain chain.
    """
    sp = np.asarray(numbers, np.int64)
    perm = np.argsort(sp, kind="stable")
    core_species = []
    halves = []
    load_max = 1
    for s in range(S):
        idx = perm[sp[perm] == s]
        h = (len(idx) + 1) // 2
        for part in (idx[:h], idx[h:]):
            core_species.append(s)
            halves.append(part)
            load_max = max(load_max, len(part))
    n_st = (load_max + 511) // 512
    stw = -(-load_max // n_st)          # ceil
    stw = min(512, ((stw + 7) // 8) * 8)
    widths = [stw] * n_st
    cap = sum(widths)
    core_idx = []
    for part in halves:
        pad = np.full(cap - len(part), -1, np.int64)
        core_idx.append(np.concatenate([part, pad]))
    return core_species, core_idx, widths


def _build_program(widths, sc1, sc2):
    import concourse.bass as bass
    import concourse.bacc as bacc
    import concourse.mybir as mybir
    from concourse import tile

    dt = mybir.dt
    nc = bacc.Bacc("TRN2", target_bir_lowering=False, debug=False,
                   enable_asserts=False, num_devices=NCORES)

    n_st = len(widths)
    wmax = max(widths)
    offs = [sum(widths[:i]) for i in range(n_st)]
    cap = sum(widths)

    xTs = [nc.dram_tensor(f"xT{i}", [128, KT1, widths[i]], dt.float8e4,
                          kind="ExternalInput") for i in range(n_st)]
    w1 = nc.dram_tensor("w1", [128, KT1, H1], dt.float8e4, kind="ExternalInput")
    w2 = nc.dram_tensor("w2", [128, 4, H2], dt.float8e4, kind="ExternalInput")
    b1 = nc.dram_tensor("b1", [128, 4], dt.float32, kind="ExternalInput")
    w3 = nc.dram_tensor("w3", [128, 4], dt.float32, kind="ExternalInput")
    ones = nc.dram_tensor("ones", [128, 1], dt.bfloat16, kind="ExternalInput")
    eo = nc.dram_tensor("eo", [1, cap], dt.float32, kind="ExternalOutput")

    silu = mybir.ActivationFunctionType.Silu
    copyf = mybir.ActivationFunctionType.Copy
    dr = mybir.MatmulPerfMode.DoubleRow
    mul = mybir.AluOpType.mult
    add = mybir.AluOpType.add

    NKP = KT1 // 2

    with tile.TileContext(nc) as tc:
        with (
            tc.tile_pool(name="wres", bufs=1) as wpool,
            tc.tile_pool(name="xs", bufs=3) as xpool,
            tc.tile_pool(name="h1p", bufs=2) as h1pool,
            tc.tile_pool(name="h2p", bufs=2) as h2pool,
            tc.tile_pool(name="zp", bufs=2) as zpool,
            tc.tile_pool(name="ps1", bufs=4, space="PSUM") as p1pool,
            tc.tile_pool(name="ps2", bufs=2, space="PSUM") as p2pool,
            tc.tile_pool(name="ps3", bufs=2, space="PSUM") as p3pool,
        ):
            ws1 = wpool.tile([128, KT1, H1], dt.float8e4, tag="ws1")
            eout = wpool.tile([1, cap], dt.float32, tag="eout")

            # PE warm-up: matmuls against a memset scratch tile (PSUM result
            # never read). With no DMA dependency they run while the input
            # data is still streaming in, so the HAM clock-gate ramps to full
            # rate before the real matmuls start.
            wrm = wpool.tile([128, 256], dt.float8e4, tag="wrm")
            nc.vector.memset(wrm[:], 0.0)

            # small tensors first (cheap, needed by the first activations)
            bs = wpool.tile([128, 4], dt.float32, tag="bs")
            nc.sync.dma_start(bs[:], b1[:])
            w3s = wpool.tile([128, 4], dt.float32, tag="w3s")
            nc.sync.dma_start(w3s[:], w3[:])
            on1 = wpool.tile([128, 1], dt.bfloat16, tag="on1")
            nc.sync.dma_start(on1[:], ones[:])

            # supertile 0 x-tile: DMA'd per k-tile-pair, interleaved with the
            # matching w1 pair so layer-1 compute starts as pairs land; the w2
            # chunks ride along mid-stream so they land before layer 2 of st0.
            w0 = widths[0]
            xs0 = xpool.tile([128, KT1, wmax], dt.float8e4, name="xs")
            for kp in range(NKP):
                nc.sync.dma_start(ws1[:, 2 * kp:2 * kp + 2, :],
                                  w1[:, 2 * kp:2 * kp + 2, :])
                nc.scalar.dma_start(xs0[:, 2 * kp:2 * kp + 2, :w0],
                                    xTs[0][:, 2 * kp:2 * kp + 2, :])
            ws2 = wpool.tile([128, 4, H2], dt.float8e4, tag="ws2")
            nc.scalar.dma_start(ws2[:], w2[:])

            h1q = []            # (st, h1) awaiting layer 2
            zq = []             # (st, z) awaiting the ones-reduce matmul
            psw = p3pool.tile([1, 512], dt.float32, name="ps3")
            for i in range(24):
                nc.tensor.matmul(psw[:, :128], wrm[:, i:i + 1], wrm[:, 128:256],
                                 start=(i == 0), stop=(i == 23))

            for st in range(n_st + 2):
                if st < n_st:
                    w = widths[st]
                    if st == 0:
                        xs = xs0
                    else:
                        xs = xpool.tile([128, KT1, wmax], dt.float8e4, name="xs")
                        for c in range(4):
                            eng = nc.sync if c % 2 == 0 else nc.scalar
                            eng.dma_start(xs[:, 5 * c:5 * c + 5, :w],
                                          xTs[st][:, 5 * c:5 * c + 5, :])
                    h1 = h1pool.tile([128, 4, wmax], dt.float8e4, name="h1")
                    if st == 0:
                        # kp-major: all 4 PSUM banks accumulate in lockstep so
                        # supertile 0 computes while x streams in.
                        pss = [p1pool.tile([128, 512], dt.float32, name="ps1")
                               for _ in range(4)]
                        for kp in range(NKP):
                            for hb in range(4):
                                nc.tensor.matmul(
                                    pss[hb][:, :w],
                                    ws1[:, 2 * kp:2 * kp + 2, hb * 128:(hb + 1) * 128],
                                    xs[:, 2 * kp:2 * kp + 2, :w],
                                    start=(kp == 0), stop=(kp == NKP - 1),
                                    perf_mode=dr)
                        for hb in range(4):
                            nc.scalar.activation(h1[:, hb, :w], pss[hb][:, :w],
                                                 silu, bias=bs[:, hb:hb + 1],
                                                 scale=sc1)
                    else:
                        # hb-major with the silu issued right after each hb
                        # block, so ScalarE drains banks while the PE works on
                        # the next block and never gates the following ST.
                        for hb in range(4):
                            ps = p1pool.tile([128, 512], dt.float32, name="ps1")
                            for kp in range(NKP):
                                nc.tensor.matmul(
                                    ps[:, :w],
                                    ws1[:, 2 * kp:2 * kp + 2, hb * 128:(hb + 1) * 128],
                                    xs[:, 2 * kp:2 * kp + 2, :w],
                                    start=(kp == 0), stop=(kp == NKP - 1),
                                    perf_mode=dr)
                            nc.scalar.activation(h1[:, hb, :w], ps[:, :w], silu,
                                                 bias=bs[:, hb:hb + 1], scale=sc1)
                    h1q.append((st, h1))
                if st > 1 and st > n_st:
                    pst, zp = zq.pop(0)
                    w = widths[pst]
                    ps3 = p3pool.tile([1, 512], dt.float32, name="ps3")
                    nc.tensor.matmul(ps3[:, :w], on1[:, 0:1], zp[:, :w])
                    nc.scalar.activation(eout[:, offs[pst]:offs[pst] + w],
                                         ps3[:, :w], copyf)
                    nc.sync.dma_start(eo[:, offs[pst]:offs[pst] + w],
                                      eout[:, offs[pst]:offs[pst] + w])
                if 0 < st <= n_st:
                    pst, h1p = h1q.pop(0)
                    w = widths[pst]
                    h2 = h2pool.tile([128, 4, wmax], dt.bfloat16, name="h2")
                    for hb in range(4):
                        ps = p2pool.tile([128, 512], dt.float32, name="ps2")
                        for kp in range(2):
                            nc.tensor.matmul(
                                ps[:, :w],
                                ws2[:, 2 * kp:2 * kp + 2, hb * 128:(hb + 1) * 128],
                                h1p[:, 2 * kp:2 * kp + 2, :w],
                                start=(kp == 0), stop=(kp == 1),
                                perf_mode=dr)
                        nc.scalar.activation(h2[:, hb, :w], ps[:, :w], silu,
                                             scale=sc2)
                    # z = sum_kt h2[:,kt,:] * w3[:,kt] on the (idle) VectorE
                    z = zpool.tile([128, wmax], dt.bfloat16, name="z")
                    nc.vector.tensor_scalar_mul(z[:, :w], h2[:, 0, :w], w3s[:, 0:1])
                    for kt in range(1, 4):
                        nc.vector.scalar_tensor_tensor(
                            z[:, :w], h2[:, kt, :w], w3s[:, kt:kt + 1], z[:, :w],
                            mul, add)
                    zq.append((pst, z))
                if st > 1 and st <= n_st:
                    pst, zp = zq.pop(0)
                    w = widths[pst]
                    ps3 = p3pool.tile([1, 512], dt.float32, name="ps3")
                    nc.tensor.matmul(ps3[:, :w], on1[:, 0:1], zp[:, :w])
                    nc.scalar.activation(eout[:, offs[pst]:offs[pst] + w],
                                         ps3[:, :w], copyf)
                    nc.sync.dma_start(eo[:, offs[pst]:offs[pst] + w],
                                      eout[:, offs[pst]:offs[pst] + w])

    nc.compile()
    return nc


def _silu(v):
    return v / (1.0 + np.exp(-v))


def _pow2_scale(absmax):
    """Largest power-of-2 k with absmax * 2**k <= FP8MAX (clamped)."""
    if absmax <= 0:
        return 0
    k = int(np.floor(np.log2(FP8MAX / absmax)))
    return max(-20, min(20, k))


def _install_trace_hook():
    """Provide antenv.axon_hooks with a ctypes NTFF hook if it's missing."""
    import sys
    import types
    import ctypes
    import contextlib
    try:
        import antenv.axon_hooks  # noqa: F401
        return
    except ImportError:
        pass
    so_path = "/opt/axon/libaxon_pjrt.so"
    if not os.path.exists(so_path):
        return
    lib = ctypes.CDLL(so_path)
    if not hasattr(lib, "axon_start_nrt_profile"):
        return
    lib.axon_start_nrt_profile.argtypes = [ctypes.POINTER(ctypes.c_int64), ctypes.c_size_t]
    lib.axon_start_nrt_profile.restype = ctypes.c_int64
    lib.axon_stop_nrt_profile.argtypes = [ctypes.c_char_p]
    lib.axon_stop_nrt_profile.restype = ctypes.c_int64

    @contextlib.contextmanager
    def _hook(output_dir, device_ids):
        import jax
        jax.devices()
        if device_ids:
            ids = (ctypes.c_int64 * len(device_ids))(*device_ids)
            rc = lib.axon_start_nrt_profile(ids, len(device_ids))
        else:
            rc = lib.axon_start_nrt_profile(None, 0)
        if rc != 0:
            raise RuntimeError(f"axon_start_nrt_profile rc={rc}")
        try:
            yield
        finally:
            n = lib.axon_stop_nrt_profile(str(output_dir).encode())
            print(f"profile: {n} file(s) written to {output_dir}")

    mod = types.ModuleType("antenv.axon_hooks")
    mod.get_axon_ntff_profile_hook = lambda: _hook
    mod.set_axon_ntff_profile_hook = lambda h: None
    import antenv
    antenv.axon_hooks = mod
    sys.modules["antenv.axon_hooks"] = mod


def kernel(positions, cells, numbers, edge_indices, edge_offsets, batch,
           U, gamma, beta, W1, W2, W3, Wc):
    global LAST_EXEC_NS
    numbers = np.asarray(numbers, np.int64)
    batch = np.asarray(batch, np.int64)
    Uf = np.asarray(U, np.float32)

    psn = _host_features(positions, numbers, edge_indices, Uf, gamma, beta)
    gamma = np.asarray(gamma, np.float32)
    beta = np.asarray(beta, np.float32)

    Wsp1 = np.einsum('as,aio->sio', Uf, np.asarray(W1, np.float32))
    Wsp2 = np.einsum('as,aio->sio', Uf, np.asarray(W2, np.float32))
    Wsp3 = np.einsum('as,aio->sio', Uf, np.asarray(W3, np.float32))

    # symmetry fold: ps[(q,p,l)] == ps[(p,q,l)]; contract unique cols only,
    # with gamma folded into W1 and beta becoming a per-hidden bias.
    qi, pi = np.triu_indices(Q)
    cols = (qi[:, None] * (Q * 3) + pi[:, None] * 3 + np.arange(3)).reshape(-1)
    swap = (pi[:, None] * (Q * 3) + qi[:, None] * 3 + np.arange(3)).reshape(-1)
    dup = np.repeat((qi != pi).astype(np.float32), 3)
    W1f = (gamma[cols, None] * Wsp1[:, cols, :]
           + dup[:, None] * gamma[swap, None] * Wsp1[:, swap, :])  # [S,FU,H1]
    b0 = np.einsum('f,sfo->so', beta, Wsp1)                        # [S,H1]

    e_atom = np.zeros(N_ATOMS, np.float32)

    if os.environ.get("KERNEL_EMULATE") == "1":
        X = psn[:, cols]
        for s in range(S):
            m = numbers == s
            hs = _silu(X[m] @ W1f[s] + b0[s])
            hs = _silu(hs @ Wsp2[s])
            e_atom[m] = (hs @ Wsp3[s])[:, 0]
    else:
        kx = _pow2_scale(np.abs(psn).max())
        k1 = _pow2_scale(np.abs(W1f).max())
        k2 = _pow2_scale(np.abs(Wsp2).max())
        sc1 = float(2.0 ** (-kx - k1))
        sc2 = float(2.0 ** (-k2))

        core_species, core_idx, widths = _plan_shards(numbers)
        n_st = len(widths)
        wmax = max(widths)
        offs = [sum(widths[:i]) for i in range(n_st)]

        fp8 = ml_dtypes.float8_e4m3fn
        bf16 = ml_dtypes.bfloat16
        # padded, quantized feature matrix; row N_ATOMS is the zero dummy row
        xq = np.zeros((N_ATOMS + 1, FP), fp8)
        xq[:N_ATOMS, :FU] = np.clip(psn[:, cols] * 2.0 ** kx,
                                    -FP8MAX, FP8MAX).astype(fp8)
        w1q = np.zeros((S, FP, H1), fp8)
        w1q[:, :FU, :] = np.clip(W1f * 2.0 ** k1, -FP8MAX, FP8MAX).astype(fp8)
        w2q = np.clip(Wsp2 * 2.0 ** k2, -FP8MAX, FP8MAX).astype(fp8)

        in_maps = []
        for ci in range(NCORES):
            s = core_species[ci]
            idx = core_idx[ci]
            idx_safe = np.where(idx < 0, N_ATOMS, idx)
            imap = {}
            for st in range(n_st):
                w = widths[st]
                blk = xq[idx_safe[offs[st]:offs[st] + w]]   # [w, FP]
                imap[f"xT{st}"] = np.ascontiguousarray(
                    blk.reshape(w, KT1, 128).transpose(2, 1, 0))
            w1_c = np.ascontiguousarray(
                w1q[s].reshape(KT1, 128, H1).transpose(1, 0, 2))
            w2_c = np.ascontiguousarray(
                w2q[s].reshape(4, 128, H2).transpose(1, 0, 2))
            b1_c = np.ascontiguousarray(b0[s].reshape(4, 128).T)
            w3_c = np.ascontiguousarray(
                Wsp3[s][:, 0].astype(np.float32).reshape(4, 128).T)
            ones_c = np.ones((128, 1), bf16)
            imap.update({"w1": w1_c, "w2": w2_c, "b1": b1_c,
                         "w3": w3_c, "ones": ones_c})
            in_maps.append(imap)

        key = (tuple(widths), sc1, sc2)
        if key not in _COMPILED:
            _COMPILED[key] = _build_program(widths, sc1, sc2)
        nc = _COMPILED[key]

        from concourse.bass_utils import run_bass_kernel_spmd
        trace = os.environ.get("KERNEL_TRACE", "0") == "1"
        if trace or os.environ.get("BASS_TRACE"):
            try:
                _install_trace_hook()
            except Exception as e:
                print(f"trace hook install failed: {e}")
        res = run_bass_kernel_spmd(nc, in_maps, core_ids=list(range(NCORES)),
                                   trace=trace)
        LAST_EXEC_NS = res.exec_time_ns
        for ci in range(NCORES):
            eo = np.asarray(res.results[ci]["eo"]).astype(np.float32)[0]
            idx = core_idx[ci]
            valid = idx >= 0
            e_atom[idx[valid]] = eo[valid]

    e_mol = np.bincount(batch, weights=e_atom.astype(np.float64),
                        minlength=N_MOL).astype(np.float32)
    e_mol = e_mol / np.sqrt(float(A)) / AVG_ATOMS
    comp = np.zeros((N_MOL, S), np.float32)
    np.add.at(comp, (batch, numbers), 1.0)
    out = e_mol[:, None] * SCALE + comp @ np.asarray(Wc, np.float32).T
    return out.astype(np.float32)


# revision 27
# speedup vs baseline: 1.0371x; 1.0371x over previous
"""Trainium2 Bass kernel for nn_AlchemicalModel (gnn_message_passing).

Strategy (v2):
  - Host (numpy): edge-basis features, per-atom spherical expansion via
    sorted segment-sum, power spectrum, layer norm, species-sorted atom
    sharding, readout.
  - Device (8 NeuronCores, SPMD): the dominant species-indexed 3-layer
    MLP. One species per core (2 cores per species) so weights are
    DMA'd once per core. Layers 1+2 run as fp8-e4m3 DoubleRow matmuls
    (two 128-K-tiles per instruction, 2x tensor throughput, half DMA),
    with power-of-2 weight scaling folded into the activation scale.
    Layer 3 (H2 -> 1) runs on-device so the output DMA is tiny.
    L2/L3 of supertile st-1 are issued after L1 of st so ScalarE silu
    latency never stalls the PE.
Self-contained: hardcodes all shapes; no sibling imports.
"""

import os
import numpy as np
import ml_dtypes

N_ATOMS = 16384
N_EDGES = 524288
N_MOL = 256
A = 4
S = 4
NMAX = 10
NSPH = 9
Q = A * NMAX
F = Q * Q * 3          # 4800
FU = 2460              # unique (l, q<=p) power-spectrum columns
FP = 2560              # FU padded to 20*128
KT1 = 20               # k-tiles for layer 1
CUTOFF = 5.0
AVG_ATOMS = 64.0
SCALE = 1.0
H1, H2 = 512, 512
NCORES = 8
FP8MAX = 240.0         # TRN fp8_e4m3 saturation

LAST_EXEC_NS = None

_COMPILED = {}


def _sph_l012(u):
    x, y, z = u[:, 0], u[:, 1], u[:, 2]
    c1 = 0.4886025119029199
    c2 = 1.0925484305920792
    return np.stack([
        np.full_like(x, 0.28209479177387814),
        c1 * y, c1 * z, c1 * x,
        c2 * x * y, c2 * y * z,
        0.31539156525252005 * (3.0 * z * z - 1.0),
        c2 * x * z,
        0.5462742152960396 * (x * x - y * y),
    ], axis=-1).astype(np.float32)


def _host_features(positions, numbers, edge_indices, U, gamma, beta):
    """Edge basis -> spherical expansion -> power spectrum -> layernorm.

    Returns x [N, F] float32 (normalized, WITHOUT gamma/beta applied).
    """
    pos = np.asarray(positions, np.float32)
    send = np.asarray(edge_indices[0], np.int64)
    recv = np.asarray(edge_indices[1], np.int64)
    rvec = pos[recv] - pos[send]                       # cells/offsets are zeros
    r = np.sqrt((rvec * rvec).sum(-1) + 1e-12).astype(np.float32)
    rhat = (rvec / r[:, None]).astype(np.float32)
    fc = (0.5 * (np.cos(np.pi * r / CUTOFF) + 1.0) * (r < CUTOFF)).astype(np.float32)
    mu = np.linspace(0.0, CUTOFF, NMAX, dtype=np.float32)
    sig = CUTOFF / NMAX
    R = np.exp(-((r[:, None] - mu) ** 2) / (2.0 * sig * sig)).astype(np.float32) * fc[:, None]
    Y = _sph_l012(rhat)                                # [E,9]
    RYf = (R[:, :, None] * Y[:, None, :]).reshape(N_EDGES, NMAX * NSPH)
    w = np.asarray(U, np.float32)[:, np.asarray(numbers, np.int64)[send]]  # [A,E]

    order = np.argsort(recv, kind="stable")
    recv_s = recv[order]
    starts = np.searchsorted(recv_s, np.arange(N_ATOMS))
    counts = np.bincount(recv, minlength=N_ATOMS)
    starts_c = np.minimum(starts, N_EDGES - 1)
    RYs = RYf[order]
    c = np.empty((N_ATOMS, A, NMAX * NSPH), np.float32)
    for a in range(A):
        z = w[a][order, None] * RYs
        ca = np.add.reduceat(z, starts_c, axis=0)
        ca[counts == 0] = 0.0
        c[:, a] = ca
    c = c.reshape(N_ATOMS, Q, NSPH)

    lblocks = [(0, 1, 1.0), (1, 4, 3.0), (4, 9, 5.0)]
    ps = np.empty((N_ATOMS, Q, Q, 3), np.float32)
    for li, (a0, b0, nl) in enumerate(lblocks):
        cb = c[:, :, a0:b0]
        ps[:, :, :, li] = np.matmul(cb, cb.transpose(0, 2, 1)) / np.sqrt(nl)
    ps = ps.reshape(N_ATOMS, F)

    mean = ps.mean(axis=-1, keepdims=True)
    var = ps.var(axis=-1, keepdims=True)
    psn = (ps - mean) / np.sqrt(var + 1e-5)
    return psn.astype(np.float32)


def _plan_shards(numbers):
    """One species per core (2 cores per species), species-sorted atoms.

    Returns (core_species[8], core_idx[8] each [sum(widths)] with -1 pad,
    widths). Widths are per-supertile atom counts: a narrow first tile so
    the pipeline fills with little DMA, 512-wide middles, and a narrow
    last tile to shorten the serial drain chain.
    """
    sp = np.asarray(numbers, np.int64)
    perm = np.argsort(sp, kind="stable")
    core_species = []
    halves = []
    load_max = 1
    for s in range(S):
        idx = perm[sp[perm] == s]
        h = (len(idx) + 1) // 2
        for part in (idx[:h], idx[h:]):
            core_species.append(s)
            halves.append(part)
            load_max = max(load_max, len(part))
    n_st = (load_max + 511) // 512
    stw = -(-load_max // n_st)          # ceil
    stw = min(512, ((stw + 7) // 8) * 8)
    widths = [stw] * n_st
    cap = sum(widths)
    core_idx = []
    for part in halves:
        pad = np.full(cap - len(part), -1, np.int64)
        core_idx.append(np.concatenate([part, pad]))
    return core_species, core_idx, widths


def _build_program(widths, sc1, sc2):
    import concourse.bass as bass
    import concourse.bacc as bacc
    import concourse.mybir as mybir
    from concourse import tile

    dt = mybir.dt
    nc = bacc.Bacc("TRN2", target_bir_lowering=False, debug=False,
                   enable_asserts=False, num_devices=NCORES)

    n_st = len(widths)
    wmax = max(widths)
    offs = [sum(widths[:i]) for i in range(n_st)]
    cap = sum(widths)

    xTs = [nc.dram_tensor(f"xT{i}", [128, KT1, widths[i]], dt.float8e4,
                          kind="ExternalInput") for i in range(n_st)]
    w1 = nc.dram_tensor("w1", [128, KT1, H1], dt.float8e4, kind="ExternalInput")
    w2 = nc.dram_tensor("w2", [128, 4, H2], dt.float8e4, kind="ExternalInput")
    b1 = nc.dram_tensor("b1", [128, 4], dt.float32, kind="ExternalInput")
    w3 = nc.dram_tensor("w3", [128, 4], dt.float32, kind="ExternalInput")
    ones = nc.dram_tensor("ones", [128, 1], dt.bfloat16, kind="ExternalInput")
    eo = nc.dram_tensor("eo", [1, cap], dt.float32, kind="ExternalOutput")

    silu = mybir.ActivationFunctionType.Silu
    copyf = mybir.ActivationFunctionType.Copy
    dr = mybir.MatmulPerfMode.DoubleRow
    mul = mybir.AluOpType.mult
    add = mybir.AluOpType.add

    NKP = KT1 // 2

    with tile.TileContext(nc) as tc:
        with (
            tc.tile_pool(name="wres", bufs=1) as wpool,
            tc.tile_pool(name="xs", bufs=3) as xpool,
            tc.tile_pool(name="h1p", bufs=2) as h1pool,
            tc.tile_pool(name="h2p", bufs=2) as h2pool,
            tc.tile_pool(name="zp", bufs=2) as zpool,
            tc.tile_pool(name="ps1", bufs=4, space="PSUM") as p1pool,
            tc.tile_pool(name="ps2", bufs=2, space="PSUM") as p2pool,
            tc.tile_pool(name="ps3", bufs=2, space="PSUM") as p3pool,
        ):
            ws1 = wpool.tile([128, KT1, H1], dt.float8e4, tag="ws1")
            eout = wpool.tile([1, cap], dt.float32, tag="eout")

            # PE warm-up: matmuls against a memset scratch tile (PSUM result
            # never read). With no DMA dependency they run while the input
            # data is still streaming in, so the HAM clock-gate ramps to full
            # rate before the real matmuls start.
            wrm = wpool.tile([128, 256], dt.float8e4, tag="wrm")
            nc.vector.memset(wrm[:], 0.0)

            # small tensors first (cheap, needed by the first activations)
            bs = wpool.tile([128, 4], dt.float32, tag="bs")
            nc.sync.dma_start(bs[:], b1[:])
            w3s = wpool.tile([128, 4], dt.float32, tag="w3s")
            nc.sync.dma_start(w3s[:], w3[:])
            on1 = wpool.tile([128, 1], dt.bfloat16, tag="on1")
            nc.sync.dma_start(on1[:], ones[:])

            # supertile 0 x-tile: DMA'd per k-tile-pair, interleaved with the
            # matching w1 pair so layer-1 compute starts as pairs land; the w2
            # chunks ride along mid-stream so they land before layer 2 of st0.
            w0 = widths[0]
            xs0 = xpool.tile([128, KT1, wmax], dt.float8e4, name="xs")
            for kp in range(NKP):
                nc.sync.dma_start(ws1[:, 2 * kp:2 * kp + 2, :],
                                  w1[:, 2 * kp:2 * kp + 2, :])
                nc.scalar.dma_start(xs0[:, 2 * kp:2 * kp + 2, :w0],
                                    xTs[0][:, 2 * kp:2 * kp + 2, :])
            ws2 = wpool.tile([128, 4, H2], dt.float8e4, tag="ws2")
            nc.scalar.dma_start(ws2[:], w2[:])

            h1q = []            # (st, h1) awaiting layer 2
            zq = []             # (st, z) awaiting the ones-reduce matmul
            psw = p3pool.tile([1, 512], dt.float32, name="ps3")
            for i in range(40):
                nc.tensor.matmul(psw[:, :128], wrm[:, i:i + 1], wrm[:, 128:256],
                                 start=(i == 0), stop=(i == 39))

            for st in range(n_st + 2):
                if st < n_st:
                    w = widths[st]
                    if st == 0:
                        xs = xs0
                    else:
                        xs = xpool.tile([128, KT1, wmax], dt.float8e4, name="xs")
                        for c in range(4):
                            eng = nc.sync if c % 2 == 0 else nc.scalar
                            eng.dma_start(xs[:, 5 * c:5 * c + 5, :w],
                                          xTs[st][:, 5 * c:5 * c + 5, :])
                    h1 = h1pool.tile([128, 4, wmax], dt.float8e4, name="h1")
                    if st == 0:
                        # kp-major: all 4 PSUM banks accumulate in lockstep so
                        # supertile 0 computes while x streams in.
                        pss = [p1pool.tile([128, 512], dt.float32, name="ps1")
                               for _ in range(4)]
                        for kp in range(NKP):
                            for hb in range(4):
                                nc.tensor.matmul(
                                    pss[hb][:, :w],
                                    ws1[:, 2 * kp:2 * kp + 2, hb * 128:(hb + 1) * 128],
                                    xs[:, 2 * kp:2 * kp + 2, :w],
                                    start=(kp == 0), stop=(kp == NKP - 1),
                                    perf_mode=dr)
                        for hb in range(4):
                            nc.scalar.activation(h1[:, hb, :w], pss[hb][:, :w],
                                                 silu, bias=bs[:, hb:hb + 1],
                                                 scale=sc1)
                    else:
                        # hb-major with the silu issued right after each hb
                        # block, so ScalarE drains banks while the PE works on
                        # the next block and never gates the following ST.
                        for hb in range(4):
                            ps = p1pool.tile([128, 512], dt.float32, name="ps1")
                            for kp in range(NKP):
                                nc.tensor.matmul(
                                    ps[:, :w],
                                    ws1[:, 2 * kp:2 * kp + 2, hb * 128:(hb + 1) * 128],
                                    xs[:, 2 * kp:2 * kp + 2, :w],
                                    start=(kp == 0), stop=(kp == NKP - 1),
                                    perf_mode=dr)
                            nc.scalar.activation(h1[:, hb, :w], ps[:, :w], silu,
                                                 bias=bs[:, hb:hb + 1], scale=sc1)
                    h1q.append((st, h1))
                if st > 1 and st > n_st:
                    pst, zp = zq.pop(0)
                    w = widths[pst]
                    ps3 = p3pool.tile([1, 512], dt.float32, name="ps3")
                    nc.tensor.matmul(ps3[:, :w], on1[:, 0:1], zp[:, :w])
                    nc.scalar.activation(eout[:, offs[pst]:offs[pst] + w],
                                         ps3[:, :w], copyf)
                    nc.sync.dma_start(eo[:, offs[pst]:offs[pst] + w],
                                      eout[:, offs[pst]:offs[pst] + w])
                if 0 < st <= n_st:
                    pst, h1p = h1q.pop(0)
                    w = widths[pst]
                    h2 = h2pool.tile([128, 4, wmax], dt.bfloat16, name="h2")
                    for hb in range(4):
                        ps = p2pool.tile([128, 512], dt.float32, name="ps2")
                        for kp in range(2):
                            nc.tensor.matmul(
                                ps[:, :w],
                                ws2[:, 2 * kp:2 * kp + 2, hb * 128:(hb + 1) * 128],
                                h1p[:, 2 * kp:2 * kp + 2, :w],
                                start=(kp == 0), stop=(kp == 1),
                                perf_mode=dr)
                        nc.scalar.activation(h2[:, hb, :w], ps[:, :w], silu,
                                             scale=sc2)
                    # z = sum_kt h2[:,kt,:] * w3[:,kt] on the (idle) VectorE
                    z = zpool.tile([128, wmax], dt.bfloat16, name="z")
                    nc.vector.tensor_scalar_mul(z[:, :w], h2[:, 0, :w], w3s[:, 0:1])
                    for kt in range(1, 4):
                        nc.vector.scalar_tensor_tensor(
                            z[:, :w], h2[:, kt, :w], w3s[:, kt:kt + 1], z[:, :w],
                            mul, add)
                    zq.append((pst, z))
                if st > 1 and st <= n_st:
                    pst, zp = zq.pop(0)
                    w = widths[pst]
                    ps3 = p3pool.tile([1, 512], dt.float32, name="ps3")
                    nc.tensor.matmul(ps3[:, :w], on1[:, 0:1], zp[:, :w])
                    nc.scalar.activation(eout[:, offs[pst]:offs[pst] + w],
                                         ps3[:, :w], copyf)
                    nc.sync.dma_start(eo[:, offs[pst]:offs[pst] + w],
                                      eout[:, offs[pst]:offs[pst] + w])

    nc.compile()
    return nc


def _silu(v):
    return v / (1.0 + np.exp(-v))


def _pow2_scale(absmax):
    """Largest power-of-2 k with absmax * 2**k <= FP8MAX (clamped)."""
    if absmax <= 0:
        return 0
    k = int(np.floor(np.log2(FP8MAX / absmax)))
    return max(-20, min(20, k))


def _install_trace_hook():
    """Provide antenv.axon_hooks with a ctypes NTFF hook if it's missing."""
    import sys
    import types
    import ctypes
    import contextlib
    try:
        import antenv.axon_hooks  # noqa: F401
        return
    except ImportError:
        pass
    so_path = "/opt/axon/libaxon_pjrt.so"
    if not os.path.exists(so_path):
        return
    lib = ctypes.CDLL(so_path)
    if not hasattr(lib, "axon_start_nrt_profile"):
        return
    lib.axon_start_nrt_profile.argtypes = [ctypes.POINTER(ctypes.c_int64), ctypes.c_size_t]
    lib.axon_start_nrt_profile.restype = ctypes.c_int64
    lib.axon_stop_nrt_profile.argtypes = [ctypes.c_char_p]
    lib.axon_stop_nrt_profile.restype = ctypes.c_int64

    @contextlib.contextmanager
    def _hook(output_dir, device_ids):
        import jax
        jax.devices()
        if device_ids:
            ids = (ctypes.c_int64 * len(device_ids))(*device_ids)
            rc = lib.axon_start_nrt_profile(ids, len(device_ids))
        else:
            rc = lib.axon_start_nrt_profile(None, 0)
        if rc != 0:
            raise RuntimeError(f"axon_start_nrt_profile rc={rc}")
        try:
            yield
        finally:
            n = lib.axon_stop_nrt_profile(str(output_dir).encode())
            print(f"profile: {n} file(s) written to {output_dir}")

    mod = types.ModuleType("antenv.axon_hooks")
    mod.get_axon_ntff_profile_hook = lambda: _hook
    mod.set_axon_ntff_profile_hook = lambda h: None
    import antenv
    antenv.axon_hooks = mod
    sys.modules["antenv.axon_hooks"] = mod


def kernel(positions, cells, numbers, edge_indices, edge_offsets, batch,
           U, gamma, beta, W1, W2, W3, Wc):
    global LAST_EXEC_NS
    numbers = np.asarray(numbers, np.int64)
    batch = np.asarray(batch, np.int64)
    Uf = np.asarray(U, np.float32)

    psn = _host_features(positions, numbers, edge_indices, Uf, gamma, beta)
    gamma = np.asarray(gamma, np.float32)
    beta = np.asarray(beta, np.float32)

    Wsp1 = np.einsum('as,aio->sio', Uf, np.asarray(W1, np.float32))
    Wsp2 = np.einsum('as,aio->sio', Uf, np.asarray(W2, np.float32))
    Wsp3 = np.einsum('as,aio->sio', Uf, np.asarray(W3, np.float32))

    # symmetry fold: ps[(q,p,l)] == ps[(p,q,l)]; contract unique cols only,
    # with gamma folded into W1 and beta becoming a per-hidden bias.
    qi, pi = np.triu_indices(Q)
    cols = (qi[:, None] * (Q * 3) + pi[:, None] * 3 + np.arange(3)).reshape(-1)
    swap = (pi[:, None] * (Q * 3) + qi[:, None] * 3 + np.arange(3)).reshape(-1)
    dup = np.repeat((qi != pi).astype(np.float32), 3)
    W1f = (gamma[cols, None] * Wsp1[:, cols, :]
           + dup[:, None] * gamma[swap, None] * Wsp1[:, swap, :])  # [S,FU,H1]
    b0 = np.einsum('f,sfo->so', beta, Wsp1)                        # [S,H1]

    e_atom = np.zeros(N_ATOMS, np.float32)

    if os.environ.get("KERNEL_EMULATE") == "1":
        X = psn[:, cols]
        for s in range(S):
            m = numbers == s
            hs = _silu(X[m] @ W1f[s] + b0[s])
            hs = _silu(hs @ Wsp2[s])
            e_atom[m] = (hs @ Wsp3[s])[:, 0]
    else:
        kx = _pow2_scale(np.abs(psn).max())
        k1 = _pow2_scale(np.abs(W1f).max())
        k2 = _pow2_scale(np.abs(Wsp2).max())
        sc1 = float(2.0 ** (-kx - k1))
        sc2 = float(2.0 ** (-k2))

        core_species, core_idx, widths = _plan_shards(numbers)
        n_st = len(widths)
        wmax = max(widths)
        offs = [sum(widths[:i]) for i in range(n_st)]

        fp8 = ml_dtypes.float8_e4m3fn
        bf16 = ml_dtypes.bfloat16
        # padded, quantized feature matrix; row N_ATOMS is the zero dummy row
        xq = np.zeros((N_ATOMS + 1, FP), fp8)
        xq[:N_ATOMS, :FU] = np.clip(psn[:, cols] * 2.0 ** kx,
                                    -FP8MAX, FP8MAX).astype(fp8)
        w1q = np.zeros((S, FP, H1), fp8)
        w1q[:, :FU, :] = np.clip(W1f * 2.0 ** k1, -FP8MAX, FP8MAX).astype(fp8)
        w2q = np.clip(Wsp2 * 2.0 ** k2, -FP8MAX, FP8MAX).astype(fp8)

        in_maps = []
        for ci in range(NCORES):
            s = core_species[ci]
            idx = core_idx[ci]
            idx_safe = np.where(idx < 0, N_ATOMS, idx)
            imap = {}
            for st in range(n_st):
                w = widths[st]
                blk = xq[idx_safe[offs[st]:offs[st] + w]]   # [w, FP]
                imap[f"xT{st}"] = np.ascontiguousarray(
                    blk.reshape(w, KT1, 128).transpose(2, 1, 0))
            w1_c = np.ascontiguousarray(
                w1q[s].reshape(KT1, 128, H1).transpose(1, 0, 2))
            w2_c = np.ascontiguousarray(
                w2q[s].reshape(4, 128, H2).transpose(1, 0, 2))
            b1_c = np.ascontiguousarray(b0[s].reshape(4, 128).T)
            w3_c = np.ascontiguousarray(
                Wsp3[s][:, 0].astype(np.float32).reshape(4, 128).T)
            ones_c = np.ones((128, 1), bf16)
            imap.update({"w1": w1_c, "w2": w2_c, "b1": b1_c,
                         "w3": w3_c, "ones": ones_c})
            in_maps.append(imap)

        key = (tuple(widths), sc1, sc2)
        if key not in _COMPILED:
            _COMPILED[key] = _build_program(widths, sc1, sc2)
        nc = _COMPILED[key]

        from concourse.bass_utils import run_bass_kernel_spmd
        trace = os.environ.get("KERNEL_TRACE", "0") == "1"
        if trace or os.environ.get("BASS_TRACE"):
            try:
                _install_trace_hook()
            except Exception as e:
                print(f"trace hook install failed: {e}")
        res = run_bass_kernel_spmd(nc, in_maps, core_ids=list(range(NCORES)),
                                   trace=trace)
        LAST_EXEC_NS = res.exec_time_ns
        for ci in range(NCORES):
            eo = np.asarray(res.results[ci]["eo"]).astype(np.float32)[0]
            idx = core_idx[ci]
            valid = idx >= 0
            e_atom[idx[valid]] = eo[valid]

    e_mol = np.bincount(batch, weights=e_atom.astype(np.float64),
                        minlength=N_MOL).astype(np.float32)
    e_mol = e_mol / np.sqrt(float(A)) / AVG_ATOMS
    comp = np.zeros((N_MOL, S), np.float32)
    np.add.at(comp, (batch, numbers), 1.0)
    out = e_mol[:, None] * SCALE + comp @ np.asarray(Wc, np.float32).T
    return out.astype(np.float32)
